# revision 7
# baseline (speedup 1.0000x reference)
"""Trainium2 Bass kernel for per-scene linear probe (moe_routing).

Computes, for full inputs:
    y[i] = x[i] @ Ws[scene[i]-1].T + bs[scene[i]-1]
    out  = relu(y[argsort_stable(scene)]) @ W1.T + b1

Strategy: on the host, stably sort rows by scene and split each scene's
rows evenly across 8 cores (padding each scene's count up to a multiple
of 1024 so every core gets the same, 128-aligned segment sizes).  Each
core then runs two dense bf16 GEMM phases:
  phase 1: y^T = relu(Ws[s]^T.T @ x^T + bs[s])  (per contiguous scene segment)
  phase 2: z   = y^T.T @ W1^T + b1
The final output is assembled on the host from contiguous per-core,
per-scene blocks (the global sorted order is preserved by construction).
"""

import os
from contextlib import ExitStack

import numpy as np
import ml_dtypes

import concourse.bacc as bacc
import concourse.mybir as mybir
import concourse.tile as tile
from concourse import bass_utils

P = 128
N_CORES = 8
HBS = 512  # phase-2 h block size (one PSUM bank of fp32)
IBS = 512  # phase-1 row block size (moving free dim)
BF16 = mybir.dt.bfloat16
F32 = mybir.dt.float32
_bf16 = ml_dtypes.bfloat16

# Stash of the last BassKernelResults so a test harness can read profiling info.
LAST_RESULT = None


def _build_nc(D, H, S, T, seg_lens):
    """Build the single-core Bass program (SPMD across cores via inputs).

    D: contraction/model dim (mult of 128); H: output dim (mult of HBS);
    S: number of scenes; T: rows per core (mult of 128, = sum(seg_lens));
    seg_lens: per-scene segment lengths (arbitrary — rows are the free dim
    in phase 1, so no alignment needed; phase 2 ignores scene boundaries).
    """
    KT = D // P
    JT = D // P
    HB = H // HBS
    WARM_MMS = 170  # ~36us of dummy PE work: covers initial DMA + HAM ramp

    nc = bacc.Bacc("TRN2", target_bir_lowering=False, debug=False)

    xT = nc.dram_tensor("xT", (D, T), BF16, kind="ExternalInput")
    WsT = nc.dram_tensor("WsT", (S, D, D), BF16, kind="ExternalInput")
    W1T = nc.dram_tensor("W1T", (D, H), BF16, kind="ExternalInput")
    bs_r = nc.dram_tensor("bs_r", (P, S, JT), F32, kind="ExternalInput")
    b1_bc = nc.dram_tensor("b1_bc", (P, H), F32, kind="ExternalInput")
    out = nc.dram_tensor("out", (T, H), F32, kind="ExternalOutput")
    yT_tmp = nc.dram_tensor("yT_tmp", (D, T), BF16)  # internal DRAM spill

    xT_r = xT[:].rearrange("(kt p) i -> p kt i", p=P)
    yT_r = yT_tmp[:].rearrange("(jt p) i -> p jt i", p=P)
    W1T_r = W1T[:].rearrange("(jt p) h -> p jt h", p=P)

    with tile.TileContext(nc) as tc, ExitStack() as ctx:
        # ---- warmup: dummy matmuls keep PE busy through the initial weight/x
        # DMAs and push the HAM clock gate to 8/8 before real work starts ----
        with tc.tile_pool(name="warm", bufs=1) as warm_pool, \
             tc.tile_pool(name="warm_ps", bufs=1, space="PSUM") as warm_ps:
            ww = warm_pool.tile([P, P], BF16)
            wx = warm_pool.tile([P, 512], BF16)
            nc.vector.memset(ww[:], 0.0)
            nc.vector.memset(wx[:], 0.0)
            wps = warm_ps.tile([P, 512], F32)
            for _ in range(WARM_MMS):
                nc.tensor.matmul(wps, ww, wx, start=True, stop=True)

        # ---- phase 1: per scene segment, y^T = relu(Ws[s]^T.T @ x^T + bs[s]) ----
        with tc.tile_pool(name="p1_w", bufs=2) as wpool, \
             tc.tile_pool(name="p1_x", bufs=3) as xpool, \
             tc.tile_pool(name="p1_y", bufs=4) as ypool, \
             tc.tile_pool(name="p1_b", bufs=1) as bpool, \
             tc.tile_pool(name="p1_ps", bufs=4, space="PSUM") as pspool:
            bs_sb = bpool.tile([P, S, JT], F32)
            nc.sync.dma_start(bs_sb[:], bs_r[:])
            seg_off = 0
            for s in range(S):
                if seg_lens[s] == 0:
                    continue
                w_sb = wpool.tile([P, KT, D], BF16, tag="wseg")
                wsT_r = WsT[s].rearrange("(kt p) j -> p kt j", p=P)
                for kt in range(KT):
                    nc.sync.dma_start(w_sb[:, kt], wsT_r[:, kt])
                i0 = 0
                while i0 < seg_lens[s]:
                    blk = min(IBS, seg_lens[s] - i0)
                    c0 = seg_off + i0
                    x_sb = xpool.tile([P, KT, IBS], BF16, tag="xblk")
                    nc.sync.dma_start(x_sb[:, :, :blk], xT_r[:, :, c0:c0 + blk])
                    for jt in range(JT):
                        ps = pspool.tile([P, IBS], F32, tag="ps1")
                        for kt in range(KT):
                            nc.tensor.matmul(
                                ps[:, :blk],
                                w_sb[:, kt, jt * P:(jt + 1) * P],
                                x_sb[:, kt, :blk],
                                start=(kt == 0),
                                stop=(kt == KT - 1),
                            )
                        y_sb = ypool.tile([P, IBS], BF16, tag="yst")
                        nc.scalar.activation(
                            y_sb[:, :blk], ps[:, :blk],
                            mybir.ActivationFunctionType.Relu,
                            bias=bs_sb[:, s, jt:jt + 1], scale=1.0,
                        )
                        nc.sync.dma_start(
                            yT_tmp[jt * P:(jt + 1) * P, c0:c0 + blk],
                            y_sb[:, :blk],
                        )
                    i0 += blk
                seg_off += seg_lens[s]

        # ---- phase 2: z = y^T.T @ W1^T + b1 ----
        with tc.tile_pool(name="p2_w1", bufs=1) as w1pool, \
             tc.tile_pool(name="p2_y", bufs=3) as y2pool, \
             tc.tile_pool(name="p2_z", bufs=4) as zpool, \
             tc.tile_pool(name="p2_b", bufs=1) as b1pool, \
             tc.tile_pool(name="p2_ps", bufs=4, space="PSUM") as ps2pool:
            w1_sb = w1pool.tile([P, JT, H], BF16)
            for jt in range(JT):
                nc.sync.dma_start(w1_sb[:, jt], W1T_r[:, jt])
            b1_sb = b1pool.tile([P, H], F32)
            nc.sync.dma_start(b1_sb[:], b1_bc[:])
            for it in range(T // P):
                yt_sb = y2pool.tile([P, JT, P], BF16, tag="ylhs")
                nc.sync.dma_start(yt_sb[:], yT_r[:, :, it * P:(it + 1) * P])
                for hb in range(HB):
                    ps = ps2pool.tile([P, HBS], F32, tag="ps2")
                    for jt in range(JT):
                        nc.tensor.matmul(
                            ps,
                            yt_sb[:, jt],
                            w1_sb[:, jt, hb * HBS:(hb + 1) * HBS],
                            start=(jt == 0),
                            stop=(jt == JT - 1),
                        )
                    z_sb = zpool.tile([P, HBS], F32, tag="zst")
                    nc.vector.tensor_tensor(
                        z_sb[:], ps[:], b1_sb[:, hb * HBS:(hb + 1) * HBS],
                        mybir.AluOpType.add,
                    )
                    nc.sync.dma_start(
                        out[it * P:(it + 1) * P, hb * HBS:(hb + 1) * HBS],
                        z_sb[:],
                    )

    nc.compile()
    return nc


def kernel(**inputs):
    global LAST_RESULT

    x = np.asarray(inputs["x"], dtype=np.float32)
    scene = np.asarray(inputs["scene_sign"])
    Ws = np.asarray(inputs["Ws"], dtype=np.float32)
    bs = np.asarray(inputs["bs"], dtype=np.float32)
    W1 = np.asarray(inputs["W1"], dtype=np.float32)
    b1 = np.asarray(inputs["b1"], dtype=np.float32)

    N, D = x.shape
    S = Ws.shape[0]
    H = W1.shape[0]

    s_idx = scene.astype(np.int64) - 1
    counts = np.bincount(s_idx, minlength=S)
    order = np.argsort(s_idx, kind="stable")
    scene_starts = np.concatenate([[0], np.cumsum(counts)])

    # per-core, per-scene row counts: exact ceil-split (equal across cores for
    # SPMD); only the core total T is padded up to a multiple of 128, with the
    # tail rows assigned to the last non-empty scene segment.
    n_s = [int(-(-c // N_CORES)) for c in counts]
    T_raw = int(sum(n_s))
    T = -(-T_raw // P) * P
    seg_lens = list(n_s)
    for s in range(S - 1, -1, -1):
        if seg_lens[s] > 0 or s == 0:
            seg_lens[s] += T - T_raw
            break
    seg_offs = np.concatenate([[0], np.cumsum(n_s)]).astype(np.int64)

    # ---- host-side input prep ----
    x_bf = x.astype(_bf16)
    WsT_h = np.ascontiguousarray(Ws.astype(_bf16).transpose(0, 2, 1))
    W1T_h = np.ascontiguousarray(W1.astype(_bf16).T)
    bs_r_h = np.ascontiguousarray(bs.reshape(S, D // P, P).transpose(2, 0, 1))
    b1_bc_h = np.ascontiguousarray(np.broadcast_to(b1, (P, H)))

    in_maps = []
    core_meta = []  # (s, k) -> (seg_row_start_in_core, count)
    for k in range(N_CORES):
        xk = np.zeros((T, D), dtype=_bf16)
        meta = []
        for s in range(S):
            lo = k * n_s[s]
            hi = min((k + 1) * n_s[s], int(counts[s]))
            cnt = max(0, hi - lo)
            if cnt:
                ids = order[scene_starts[s] + lo: scene_starts[s] + hi]
                xk[seg_offs[s]:seg_offs[s] + cnt] = x_bf[ids]
            meta.append(cnt)
        core_meta.append(meta)
        in_maps.append({
            "xT": np.ascontiguousarray(xk.T),
            "WsT": WsT_h,
            "W1T": W1T_h,
            "bs_r": bs_r_h,
            "b1_bc": b1_bc_h,
        })

    nc = _build_nc(D, H, S, T, seg_lens)

    trace = bool(int(os.environ.get("BASS_KERNEL_TRACE", "0")))
    res = bass_utils.run_bass_kernel_spmd(
        nc, in_maps, core_ids=list(range(N_CORES)), trace=trace,
    )
    LAST_RESULT = res

    # ---- host-side output assembly (global sorted order = concat of blocks) ----
    final = np.empty((N, H), dtype=np.float32)
    pos = 0
    for s in range(S):
        for k in range(N_CORES):
            cnt = core_meta[k][s]
            if cnt:
                zk = res.results[k]["out"]
                final[pos:pos + cnt] = zk[seg_offs[s]:seg_offs[s] + cnt]
                pos += cnt
    assert pos == N
    return final
